# revision 1
# baseline (speedup 1.0000x reference)
"""Trainium2 Bass kernel for nn_Example1 (last-row one-hot attention).

Mathematical reduction: the reference builds one-hot X from token_ids, forms
causal attention A = softmax(X R X^T + mask) and returns (A @ X)[:, -1, :].
Only the last row of A matters, and its mask row is all-zero.  With
t = token_ids[b], q = t[-1]:

    s_j  = R[q, t_j]
    a    = softmax(s)                       (no mask on the last row)
    out[w] = sum_{j: t_j == w} a_j

Since a_j depends on j only through the token value t_j, tokens with equal
value share one weight, so with count[w] = histogram(t):

    out = count * exp(R[q, :]) / <count, exp(R[q, :])>

(exp without max-subtraction is safe: R ~ N(0,1)/4096 so |s| < ~1.5e-3).

Device work per core (2 batches, data-parallel over batch across 8 cores,
R replicated):
  - indirect-DMA gather of rows R[q_b, :]
  - token histogram into 4096 bins via h/l factorization:
    t = 32*h + l, [t==w] = [h==wh]*[l==wl]; count2d[wh,wl] via 8 PE matmuls
    of 128x128 (one-hot(h)) x 128x32 (one-hot(l)) per batch
  - exp on ACT, dot + broadcast normalization via tiny ones-matmuls
"""

import numpy as np

import concourse.bacc as bacc
import concourse.mybir as mybir
from concourse.bass import IndirectOffsetOnAxis
from concourse.tile import TileContext

B, N, V = 16, 1024, 4096
NCORES = 8
BL = B // NCORES          # batches per core
P = 128                   # SBUF partitions
MB = N // P               # 8 j-blocks per batch (j = 8p + m)
WH, WL = 128, 32          # V = WH * WL, w = 32*wh + wl
CM = BL * MB              # (b, m) column groups in the pm layout

f32 = mybir.dt.float32
bf16 = mybir.dt.bfloat16
i32 = mybir.dt.int32
OP = mybir.AluOpType


def emit_iteration(nc, pool, psum, dram, consts, T, R, O):
    io_wh, io_wl, one_c, one_r = consts

    q_sb = pool.tile([BL, 1], i32, tag="q_sb")
    t_pm = pool.tile([P, CM], i32, tag="t_pm")
    h_i = pool.tile([P, CM], i32, tag="h_i")
    l_i = pool.tile([P, CM], i32, tag="l_i")
    h_bf = pool.tile([P, CM], bf16, tag="h_bf")
    l_bf = pool.tile([P, CM], bf16, tag="l_bf")
    Hm = pool.tile([P, CM * WH], bf16, tag="Hm")
    Vm = pool.tile([P, CM * WL], bf16, tag="Vm")
    rq_sb = pool.tile([BL, V], f32, tag="rq_sb")
    rq2d = pool.tile([P, BL * WL], f32, tag="rq2d")
    e_sb = pool.tile([P, BL * WL], f32, tag="e_sb")
    num_sb = pool.tile([P, BL * WL], f32, tag="num_sb")
    znum = pool.tile([P, BL], f32, tag="znum")
    zs = pool.tile([1, BL], f32, tag="zs")
    zinv = pool.tile([P, BL], f32, tag="zinv")
    out_sb = pool.tile([P, BL * WL], f32, tag="out_sb")
    RQ = dram.tile([BL, V], f32, tag="RQ")

    c_ps = psum.tile([P, BL * WL], f32, tag="c_ps")
    z_ps = psum.tile([1, BL], f32, tag="z_ps")
    zr_ps = psum.tile([P, BL], f32, tag="zr_ps")

    # ---- loads ----
    nc.sync.dma_start(out=q_sb[:, :], in_=T[:, N - 1 : N])
    # t_pm[p, (b, m)] = T[b, 8p + m]
    nc.sync.dma_start(
        out=t_pm[:, :].rearrange("p (b m) -> p b m", b=BL),
        in_=T[:, :].rearrange("b (p m) -> p b m", p=P),
    )
    # rq_sb[b, :] = R[q_b, :]
    nc.gpsimd.indirect_dma_start(
        out=rq_sb[:, :],
        out_offset=None,
        in_=R[:, :],
        in_offset=IndirectOffsetOnAxis(ap=q_sb[:, 0:1], axis=0),
    )
    # bounce through DRAM to relayout (2, 4096) -> (128, (b, 32))
    nc.sync.dma_start(out=RQ[:, :], in_=rq_sb[:, :])
    nc.sync.dma_start(
        out=rq2d[:, :].rearrange("p (b l) -> p b l", b=BL),
        in_=RQ[:, :].rearrange("b (p l) -> p b l", p=P),
    )

    # ---- h/l decomposition (int32), then cast to bf16 (values < 128) ----
    nc.vector.tensor_scalar(out=h_i[:, :], in0=t_pm[:, :], scalar1=5,
                            scalar2=None, op0=OP.logical_shift_right)
    nc.vector.tensor_scalar(out=l_i[:, :], in0=t_pm[:, :], scalar1=31,
                            scalar2=None, op0=OP.bitwise_and)
    nc.vector.tensor_copy(out=h_bf[:, :], in_=h_i[:, :])
    nc.vector.tensor_copy(out=l_bf[:, :], in_=l_i[:, :])

    # ---- one-hot builds ----
    nc.vector.tensor_tensor(
        out=Hm[:, :].rearrange("p (c w) -> p c w", w=WH),
        in0=h_bf[:, :, None].broadcast_to((P, CM, WH)),
        in1=io_wh[:, None, :].broadcast_to((P, CM, WH)),
        op=OP.is_equal,
    )
    nc.vector.tensor_tensor(
        out=Vm[:, :].rearrange("p (c w) -> p c w", w=WL),
        in0=l_bf[:, :, None].broadcast_to((P, CM, WL)),
        in1=io_wl[:, None, :].broadcast_to((P, CM, WL)),
        op=OP.is_equal,
    )

    # ---- histogram: count2d[wh, (b, wl)] ----
    for b in range(BL):
        for m in range(MB):
            c = b * MB + m
            nc.tensor.matmul(
                out=c_ps[:, b * WL : (b + 1) * WL],
                lhsT=Hm[:, c * WH : (c + 1) * WH],
                rhs=Vm[:, c * WL : (c + 1) * WL],
                start=(m == 0),
                stop=(m == MB - 1),
            )

    # ---- numerator and normalization ----
    nc.scalar.activation(out=e_sb[:, :], in_=rq2d[:, :],
                         func=mybir.ActivationFunctionType.Exp)
    nc.vector.tensor_tensor(out=num_sb[:, :], in0=c_ps[:, :],
                            in1=e_sb[:, :], op=OP.mult)
    nc.vector.tensor_reduce(
        out=znum[:, :],
        in_=num_sb[:, :].rearrange("p (b l) -> p b l", b=BL),
        axis=mybir.AxisListType.X,
        op=OP.add,
    )
    # Z_b = sum_p znum[p, b]; replicate back to all partitions
    nc.tensor.matmul(out=z_ps[:, :], lhsT=one_c[:, :], rhs=znum[:, :],
                     start=True, stop=True)
    nc.scalar.copy(out=zs[:, :], in_=z_ps[:, :])
    nc.tensor.matmul(out=zr_ps[:, :], lhsT=one_r[:, :], rhs=zs[:, :],
                     start=True, stop=True)
    nc.vector.reciprocal(out=zinv[:, :], in_=zr_ps[:, :])
    nc.vector.tensor_tensor(
        out=out_sb[:, :].rearrange("p (b l) -> p b l", b=BL),
        in0=num_sb[:, :].rearrange("p (b l) -> p b l", b=BL),
        in1=zinv[:, :, None].broadcast_to((P, BL, WL)),
        op=OP.mult,
    )
    nc.sync.dma_start(
        out=O[:, :].rearrange("b (p l) -> p b l", p=P),
        in_=out_sb[:, :].rearrange("p (b l) -> p b l", b=BL),
    )


def build_nc(iters: int = 1):
    nc = bacc.Bacc(trn_type="TRN2")
    T = nc.dram_tensor("token_ids", [BL, N], i32, kind="ExternalInput")
    R = nc.dram_tensor("R", [V, V], f32, kind="ExternalInput")
    O = nc.dram_tensor("out", [BL, V], f32, kind="ExternalOutput")

    with TileContext(nc) as tc:
        with tc.tile_pool(name="const", bufs=1) as cpool, \
             tc.tile_pool(name="sb", bufs=2) as pool, \
             tc.tile_pool(name="dram", bufs=2, space="DRAM") as dram, \
             tc.tile_pool(name="ps", bufs=2, space="PSUM") as psum:
            io_wh = cpool.tile([P, WH], bf16)
            io_wl = cpool.tile([P, WL], bf16)
            one_c = cpool.tile([P, 1], f32)
            one_r = cpool.tile([1, P], f32)
            nc.gpsimd.iota(io_wh[:, :], pattern=[[1, WH]], base=0,
                           channel_multiplier=0,
                           allow_small_or_imprecise_dtypes=True)
            nc.gpsimd.iota(io_wl[:, :], pattern=[[1, WL]], base=0,
                           channel_multiplier=0,
                           allow_small_or_imprecise_dtypes=True)
            nc.vector.memset(one_c[:, :], 1.0)
            nc.vector.memset(one_r[:, :], 1.0)
            consts = (io_wh, io_wl, one_c, one_r)

            for _ in range(iters):
                emit_iteration(nc, pool, psum, dram, consts, T, R, O)
    nc.finalize()
    return nc


_CACHE = {}


def _get_nc():
    if "nc" not in _CACHE:
        _CACHE["nc"] = build_nc()
    return _CACHE["nc"]


def kernel(**inputs) -> np.ndarray:
    import os

    token_ids = np.ascontiguousarray(np.asarray(inputs["token_ids"]).astype(np.int32))
    R = np.ascontiguousarray(np.asarray(inputs["R"], dtype=np.float32))
    assert token_ids.shape == (B, N) and R.shape == (V, V)

    from concourse.bass_utils import run_bass_kernel_spmd

    nc = _get_nc()
    in_maps = [
        {"token_ids": token_ids[c * BL : (c + 1) * BL], "R": R}
        for c in range(NCORES)
    ]
    res = run_bass_kernel_spmd(nc, in_maps, core_ids=list(range(NCORES)))
    _CACHE["last_results"] = res
    return np.concatenate([res.results[c]["out"] for c in range(NCORES)], axis=0)


if __name__ == "__main__":
    t = np.random.randint(0, V, size=(B, N)).astype(np.int32)
    R = (np.random.randn(V, V) / V).astype(np.float32)
    out = kernel(token_ids=t, R=R)
    print(out.shape, out.dtype, out.sum(axis=1)[:4])



# revision 7
# speedup vs baseline: 1.2477x; 1.2477x over previous
"""Trainium2 Bass kernel for nn_Example1 (last-row one-hot attention).

Mathematical reduction: the reference builds one-hot X from token_ids, forms
causal attention A = softmax(X R X^T + mask) and returns (A @ X)[:, -1, :].
Only the last row of A matters, and its mask row is all-zero.  With
t = token_ids[b], q = t[-1]:

    s_j  = R[q, t_j]
    a    = softmax(s)                       (no mask on the last row)
    out[w] = sum_{j: t_j == w} a_j

Since a_j depends on j only through the token value t_j, tokens with equal
value share one weight, so with count[w] = histogram(t):

    out = count * exp(R[q, :]) / <count, exp(R[q, :])>

Device work per core (2 batches, data-parallel over batch across 8 cores,
R replicated).  Layout: w = 64*h + l; everything lives on a [128, 64] grid
with partition index b*64+h and free index l, so the gathered row of R, the
histogram and the output all align with no transposes or bounces:

  - q replicated to all partitions by a single step-0 broadcast DMA read;
    per-partition slice index idx[p] = q[p>>6]*64 + (p&63) gathers R (viewed
    as [(v h), l]) straight into the [(b h), l] layout
  - token histogram via h/l one-hot factorization: 16 PE matmuls of
    (128 x 64 one-hot(h)) x (128 x 64 one-hot(l)), batch b selecting the
    PSUM partition block b*64..b*64+64
  - exp on ACT; fused multiply+reduce on DVE; block-diagonal ones-matmul
    for the softmax denominator; per-partition-scale normalize on ACT
"""

import numpy as np

import concourse.bacc as bacc
import concourse.mybir as mybir
from concourse.bass import IndirectOffsetOnAxis
from concourse.tile import TileContext

B, N, V = 16, 1024, 4096
NCORES = 8
BL = B // NCORES          # batches per core
P = 128                   # SBUF partitions
MB = N // P               # 8 j-blocks per batch (j = 8p + m)
W = 64                    # V = W * W, w = 64*h + l
CM = BL * MB              # (b, m) column groups in the pm layout

f32 = mybir.dt.float32
bf16 = mybir.dt.bfloat16
i32 = mybir.dt.int32
OP = mybir.AluOpType


def emit_iteration(nc, pool, psum, consts, T, R, O):
    io64, mblk, pl = consts

    q_rep = pool.tile([P, 1], i32, tag="q_rep")
    idx = pool.tile([P, 1], i32, tag="idx")
    t_pm = pool.tile([P, CM], i32, tag="t_pm")
    h_i = pool.tile([P, CM], i32, tag="h_i")
    l_i = pool.tile([P, CM], i32, tag="l_i")
    Hm = pool.tile([P, CM * W], bf16, tag="Hm")
    Vm = pool.tile([P, CM * W], bf16, tag="Vm")
    rq2d = pool.tile([P, W], f32, tag="rq2d")
    e_sb = pool.tile([P, W], f32, tag="e_sb")
    num_sb = pool.tile([P, W], f32, tag="num_sb")
    znum = pool.tile([P, 1], f32, tag="znum")
    zinv = pool.tile([P, 1], f32, tag="zinv")
    out_sb = pool.tile([P, W], f32, tag="out_sb")

    c_ps = psum.tile([P, W], f32, tag="c_ps")
    zr_ps = psum.tile([P, 1], f32, tag="zr_ps")

    # ---- loads: q on the scalar HWDGE queue, tokens on sync, in parallel ----
    # q_rep[p] = T[p >> 6, N-1]: replicated read via step-0 mid dim
    nc.scalar.dma_start(
        out=q_rep[:, :],
        in_=T[:, N - 1 : N].broadcast_to((BL, W, 1)),
    )
    # t_pm[p, (b, m)] = T[b, 8p + m]
    nc.sync.dma_start(
        out=t_pm[:, :].rearrange("p (b m) -> p b m", b=BL),
        in_=T[:, :].rearrange("b (p m) -> p b m", p=P),
    )
    # idx[p] = q_rep[p]*64 + (p & 63); gather R (viewed [(v h), l]) so that
    # rq2d[b*64 + h, l] = R[q_b, 64*h + l]
    nc.vector.tensor_scalar(out=q_rep[:, :], in0=q_rep[:, :], scalar1=6,
                            scalar2=None, op0=OP.logical_shift_left)
    nc.vector.tensor_tensor(out=idx[:, :], in0=q_rep[:, :], in1=pl[:, :],
                            op=OP.add)
    nc.gpsimd.indirect_dma_start(
        out=rq2d[:, :],
        out_offset=None,
        in_=R[:, :].rearrange("v (h l) -> (v h) l", h=W),
        in_offset=IndirectOffsetOnAxis(ap=idx[:, 0:1], axis=0),
    )

    # ---- h/l decomposition (bitVec ops can't cast, so stay in i32) ----
    nc.vector.tensor_scalar(out=h_i[:, :], in0=t_pm[:, :], scalar1=6,
                            scalar2=None, op0=OP.logical_shift_right)
    nc.vector.tensor_scalar(out=l_i[:, :], in0=t_pm[:, :], scalar1=63,
                            scalar2=None, op0=OP.bitwise_and)

    # ---- one-hot builds (i32 compare, cast to bf16 on write) ----
    nc.vector.tensor_tensor(
        out=Hm[:, :].rearrange("p (c w) -> p c w", w=W),
        in0=h_i[:, :, None].broadcast_to((P, CM, W)),
        in1=io64[:, None, :].broadcast_to((P, CM, W)),
        op=OP.is_equal,
    )
    nc.vector.tensor_tensor(
        out=Vm[:, :].rearrange("p (c w) -> p c w", w=W),
        in0=l_i[:, :, None].broadcast_to((P, CM, W)),
        in1=io64[:, None, :].broadcast_to((P, CM, W)),
        op=OP.is_equal,
    )

    # ---- histogram: c_ps[b*64 + h, l] = count_b[64*h + l] ----
    for b in range(BL):
        for m in range(MB):
            c = b * MB + m
            nc.tensor.matmul(
                out=c_ps[b * W : (b + 1) * W, :],
                lhsT=Hm[:, c * W : (c + 1) * W],
                rhs=Vm[:, c * W : (c + 1) * W],
                start=(m == 0),
                stop=(m == MB - 1),
            )

    # ---- numerator and normalization ----
    nc.scalar.activation(out=e_sb[:, :], in_=rq2d[:, :],
                         func=mybir.ActivationFunctionType.Exp)
    # num = count * e;  znum[p] = sum_l num[p, l]
    # (tensor_tensor_reduce crashes at runtime on this toolchain; use
    # separate mult + reduce)
    nc.vector.tensor_tensor(out=num_sb[:, :], in0=c_ps[:, :],
                            in1=e_sb[:, :], op=OP.mult)
    nc.vector.tensor_reduce(
        out=znum[:, :], in_=num_sb[:, :],
        axis=mybir.AxisListType.X, op=OP.add,
    )
    # Z_b broadcast to b's partition block via block-diagonal ones matmul
    nc.tensor.matmul(out=zr_ps[:, :], lhsT=mblk[:, :], rhs=znum[:, :],
                     start=True, stop=True)
    nc.vector.reciprocal(out=zinv[:, :], in_=zr_ps[:, :])
    # out = num * (1/Z_b), per-partition scale on the ACT engine
    nc.scalar.mul(out_sb[:, :], num_sb[:, :], zinv[:, 0:1])
    nc.sync.dma_start(
        out=O[:, :].rearrange("b (h l) -> (b h) l", h=W),
        in_=out_sb[:, :],
    )


def build_nc(iters: int = 1):
    nc = bacc.Bacc(trn_type="TRN2")
    T = nc.dram_tensor("token_ids", [BL, N], i32, kind="ExternalInput")
    R = nc.dram_tensor("R", [V, V], f32, kind="ExternalInput")
    O = nc.dram_tensor("out", [BL, V], f32, kind="ExternalOutput")

    with TileContext(nc) as tc:
        with tc.tile_pool(name="const", bufs=1) as cpool, \
             tc.tile_pool(name="sb", bufs=2) as pool, \
             tc.tile_pool(name="ps", bufs=2, space="PSUM") as psum:
            io64 = cpool.tile([P, W], i32)
            mblk = cpool.tile([P, P], f32)
            pl = cpool.tile([P, 1], i32)
            nc.gpsimd.iota(io64[:, :], pattern=[[1, W]], base=0,
                           channel_multiplier=0)
            nc.gpsimd.iota(pl[:, :], pattern=[[0, 1]], base=0,
                           channel_multiplier=1)
            nc.vector.tensor_scalar(out=pl[:, :], in0=pl[:, :], scalar1=63,
                                    scalar2=None, op0=OP.bitwise_and)
            # block-diagonal ones: mblk[p, i] = 1 iff p//64 == i//64
            nc.vector.memset(mblk[0:W, 0:W], 1.0)
            nc.vector.memset(mblk[0:W, W:P], 0.0)
            nc.vector.memset(mblk[W:P, 0:W], 0.0)
            nc.vector.memset(mblk[W:P, W:P], 1.0)
            consts = (io64, mblk, pl)

            for _ in range(iters):
                emit_iteration(nc, pool, psum, consts, T, R, O)
    nc.finalize()
    return nc


_CACHE = {}


def _get_nc():
    if "nc" not in _CACHE:
        _CACHE["nc"] = build_nc()
    return _CACHE["nc"]


def kernel(**inputs) -> np.ndarray:
    token_ids = np.ascontiguousarray(np.asarray(inputs["token_ids"]).astype(np.int32))
    R = np.ascontiguousarray(np.asarray(inputs["R"], dtype=np.float32))
    assert token_ids.shape == (B, N) and R.shape == (V, V)

    from concourse.bass_utils import run_bass_kernel_spmd

    nc = _get_nc()
    in_maps = [
        {"token_ids": token_ids[c * BL : (c + 1) * BL], "R": R}
        for c in range(NCORES)
    ]
    res = run_bass_kernel_spmd(nc, in_maps, core_ids=list(range(NCORES)))
    _CACHE["last_results"] = res
    return np.concatenate([res.results[c]["out"] for c in range(NCORES)], axis=0)


if __name__ == "__main__":
    t = np.random.randint(0, V, size=(B, N)).astype(np.int32)
    R = (np.random.randn(V, V) / V).astype(np.float32)
    out = kernel(token_ids=t, R=R)
    print(out.shape, out.dtype, out.sum(axis=1)[:4])
